# revision 7
# baseline (speedup 1.0000x reference)
"""Conv2d 3x3 (stride 1, pad 1) Trainium2 Bass kernel — 1D Winograd F(2,3).

Problem: x (32, 128, 56, 56) fp32, kernels (256, 128, 3, 3) fp32, b (256,) fp32
-> out (32, 256, 56, 56) fp32.

Strategy:
  - Data-parallel over batch: 32 images / 8 cores = 4 images per core. SPMD,
    no collectives.
  - Winograd F(2,3) applied along H (output rows in pairs): cuts tensor-engine
    flops to 2/3 of direct conv. Per output-row pair t, position p in 0..3:
      V_p[t] = B^T-combo of input rows 2t-1..2t+2 (computed on VectorE, bf16)
      M_p    = sum_kw sum_cin U_p[kw] * V_p[t, w+kw-1]   (PE, PSUM accum)
      out[2t]   = M_0 + M_1 + M_2 + b
      out[2t+1] = M_1 - M_2 - M_3 + b
    U_p[kw] = G-transform of the 3x3 weights along kh, precomputed on host.
  - Per (14-row block, cout half): 12 matmuls [128cin x 128cout x 392free]
    into 4 PSUM banks (one per position), double-buffered = 8 banks. kw=1
    (full window) goes first with start=True; kw=0/2 write ragged col windows.
  - Inverse transform + bias split across the idle engines:
      ScalarE: s1 = M1 + b, s2 = M2       (ACTIVATE, PSUM->SBUF bf16)
      VectorE: tA = M0 + s1, tB = s2 + M3 (tensor_tensor, one PSUM operand)
      GpSimd:  o0 = tA + s2 -> even rows, o1 = s1 - tB -> odd rows (SBUF only)
  - Output stored bf16 (halves store traffic); host casts to fp32.
"""

import numpy as np
import ml_dtypes

import concourse.bass as bass
import concourse.tile as tile
from concourse import bacc, mybir
from concourse.alu_op_type import AluOpType
from concourse.bass_utils import run_bass_kernel_spmd

N_CORES = 8
N_FULL = 32
N_PER = N_FULL // N_CORES  # 4 images per core
C_IN = 128
C_OUT = 256
H = W = 56
T = H // 2          # 28 row-pair tiles
BLK = 7             # tiles per block -> 14 output rows
NB = T // BLK       # 4 blocks
NFREE = BLK * W     # 392 fp32 <= 512 (one PSUM bank)

_DT = mybir.dt.bfloat16


def _build():
    nc = bacc.Bacc(
        "TRN2",
        target_bir_lowering=False,
        debug=False,
        num_devices=N_CORES,
    )
    xs = nc.dram_tensor("xs", [N_PER, C_IN, H, W], _DT, kind="ExternalInput").ap()
    # U layout: [cin, (p, kw, cout)] -> [128, 4*3*256]
    ut = nc.dram_tensor("ut", [C_IN, 12 * C_OUT], _DT, kind="ExternalInput").ap()
    bt = nc.dram_tensor("bt", [128, 2], mybir.dt.float32, kind="ExternalInput").ap()
    # output half-major: [n, half, cout_local, h*w]
    y = nc.dram_tensor(
        "y", [N_PER, 2, 128, H * W], _DT, kind="ExternalOutput"
    ).ap()

    with tile.TileContext(nc) as tc:
        with (
            tc.tile_pool(name="const", bufs=1) as const,
            tc.tile_pool(name="xpool", bufs=3) as xpool,
            tc.tile_pool(name="vpool", bufs=3) as vpool,
            tc.tile_pool(name="pspool", bufs=8, space="PSUM") as pspool,
            tc.tile_pool(name="evpool", bufs=3) as evpool,
            tc.tile_pool(name="opool", bufs=4) as opool,
        ):
            # PE warm-up: dummy matmuls lift the HAM clock gate (1.2 -> 2.4
            # GHz) during the input-load window.
            warm = const.tile([128, NFREE], _DT)
            nc.vector.memset(warm[:], 0.0)
            wps = pspool.tile([128, NFREE], mybir.dt.float32, tag="ps")
            N_WARM = 6
            for i in range(N_WARM):
                nc.tensor.matmul(
                    wps[:],
                    lhsT=warm[:, :128],
                    rhs=warm[:],
                    start=(i == 0),
                    stop=(i == N_WARM - 1),
                )

            # weights: one SBUF tile per Winograd position, split over the
            # scalar and gpsimd DMA queues so all parts land early
            up = []
            for p in range(4):
                t_ = const.tile([C_IN, 3 * C_OUT], _DT, name=f"u_sb{p}")
                eng = nc.scalar if p % 2 == 0 else nc.gpsimd
                eng.dma_start(out=t_[:], in_=ut[:, p * 3 * C_OUT : (p + 1) * 3 * C_OUT])
                up.append(t_)
            bias_sb = const.tile([128, 2], mybir.dt.float32)
            nc.scalar.dma_start(out=bias_sb[:], in_=bt)

            xts = [None] * N_PER
            vts = [None] * N_PER

            def load_x(n):
                xt = xpool.tile([C_IN, H, W], _DT, tag="xt", name=f"x{n}")
                nc.sync.dma_start(out=xt[:, 0:16, :], in_=xs[n, :, 0:16, :])
                nc.sync.dma_start(out=xt[:, 16:30, :], in_=xs[n, :, 16:30, :])
                nc.sync.dma_start(out=xt[:, 30:56, :], in_=xs[n, :, 30:56, :])
                xts[n] = xt
                vts[n] = vpool.tile([C_IN, 4, T, W], _DT, tag="vt", name=f"v{n}")

            def vops(n, a, b):
                # V transform on VectorE (bf16, innermost step-1 -> 2x mode)
                # for tile range [a, b):
                #   v0[t] = x[2t-1] - x[2t+1]   (t=0: -x[1])
                #   v1[t] = x[2t]   + x[2t+1]
                #   v2[t] = x[2t+1] - x[2t]
                #   v3[t] = x[2t]   - x[2t+2]   (t=27: x[54])
                xt, vt = xts[n], vts[n]
                a0 = a
                if a == 0:
                    nc.vector.tensor_scalar_mul(vt[:, 0, 0:1, :], xt[:, 1:2, :], -1.0)
                    a0 = 1
                nc.vector.tensor_sub(
                    vt[:, 0, a0:b, :], xt[:, 2 * a0 - 1 : 2 * b - 1 : 2, :],
                    xt[:, 2 * a0 + 1 : min(2 * b + 1, H) : 2, :],
                )
                nc.vector.tensor_add(
                    vt[:, 1, a:b, :], xt[:, 2 * a : 2 * b : 2, :],
                    xt[:, 2 * a + 1 : min(2 * b + 1, H) : 2, :],
                )
                nc.vector.tensor_sub(
                    vt[:, 2, a:b, :], xt[:, 2 * a + 1 : min(2 * b + 1, H) : 2, :],
                    xt[:, 2 * a : 2 * b : 2, :],
                )
                b3 = min(b, T - 1)
                nc.vector.tensor_sub(
                    vt[:, 3, a:b3, :], xt[:, 2 * a : 2 * b3 : 2, :],
                    xt[:, 2 * a + 2 : 2 * b3 + 2 : 2, :],
                )
                if b == T:
                    nc.vector.tensor_copy(vt[:, 3, T - 1 : T, :], xt[:, 54:55, :])

            load_x(0)
            vops(0, 0, BLK)
            for n in range(N_PER):
                vt = vts[n]
                for half in range(2):
                    for blk in range(NB):
                        u = half * NB + blk  # unit index 0..7 within image
                        # paced prefetch: next image's x DMAs and V chunks
                        # emitted mid-stream so DVE bursts don't stall units
                        if n == 0 and u < 3:
                            vops(0, (u + 1) * BLK, (u + 2) * BLK)
                        if n + 1 < N_PER:
                            if u == 1:
                                load_x(n + 1)
                            elif 3 <= u <= 6:
                                vops(n + 1, (u - 3) * BLK, (u - 2) * BLK)
                        t0 = blk * BLK
                        ps = [
                            pspool.tile(
                                [128, NFREE],
                                mybir.dt.float32,
                                tag="ps",
                                name=f"ps{n}_{half}_{blk}_{p}",
                            )
                            for p in range(4)
                        ]
                        for p in range(4):
                            ps3 = ps[p][:].rearrange("q (t w) -> q t w", t=BLK)
                            for kw in (1, 0, 2):
                                dw = kw - 1
                                wlo = max(0, -dw)
                                whi = W - max(0, dw)
                                nc.tensor.matmul(
                                    ps3[:, :, wlo:whi],
                                    lhsT=up[p][
                                        :, kw * C_OUT + half * 128 : kw * C_OUT + half * 128 + 128
                                    ],
                                    rhs=vt[:, p, t0 : t0 + BLK, wlo + dw : whi + dw],
                                    start=(kw == 1),
                                    stop=(kw == 2),
                                )
                        # inverse transform + bias:
                        #   o0 = m0+m1+m2+b = (s1+s2) + m0
                        #   o1 = m1-m2-m3+b = (s1-s2) + (-m3)
                        s1 = evpool.tile([128, NFREE], _DT, tag="s1")
                        s2 = evpool.tile([128, NFREE], _DT, tag="s2")
                        c3 = evpool.tile([128, NFREE], _DT, tag="c3")
                        w_ = evpool.tile([128, NFREE], _DT, tag="w_")
                        d_ = evpool.tile([128, NFREE], _DT, tag="d_")
                        nc.scalar.activation(
                            s1[:],
                            ps[1][:],
                            mybir.ActivationFunctionType.Identity,
                            bias=bias_sb[:, half : half + 1],
                            scale=1.0,
                        )
                        nc.scalar.activation(
                            s2[:],
                            ps[2][:],
                            mybir.ActivationFunctionType.Identity,
                            bias=0.0,
                            scale=1.0,
                        )
                        nc.scalar.activation(
                            c3[:],
                            ps[3][:],
                            mybir.ActivationFunctionType.Identity,
                            bias=0.0,
                            scale=-1.0,
                        )
                        ot = opool.tile([128, 2 * NFREE], _DT, tag="ot")
                        ot3 = ot[:].rearrange("q (r w) -> q r w", r=2 * BLK)
                        w3 = w_[:].rearrange("q (t w) -> q t w", t=BLK)
                        d3 = d_[:].rearrange("q (t w) -> q t w", t=BLK)
                        ps03 = ps[0][:].rearrange("q (t w) -> q t w", t=BLK)
                        c33 = c3[:].rearrange("q (t w) -> q t w", t=BLK)
                        nc.vector.tensor_add(w_[:], s1[:], s2[:])
                        nc.vector.tensor_add(ot3[:, 0 : 2 * BLK : 2, :], w3[:], ps03[:])
                        nc.gpsimd.tensor_sub(d_[:], s1[:], s2[:])
                        nc.gpsimd.tensor_add(ot3[:, 1 : 2 * BLK : 2, :], d3[:], c33[:])
                        y_slice = y[
                            n, half, :, blk * 2 * NFREE : (blk + 1) * 2 * NFREE
                        ]
                        if n == N_PER - 1 and half == 1 and blk == NB - 1:
                            # split the final store across two queues so its
                            # drain doesn't gate the end-of-kernel barrier
                            hf = NFREE
                            nc.sync.dma_start(out=y_slice[:, :hf], in_=ot[:, :hf])
                            nc.scalar.dma_start(out=y_slice[:, hf:], in_=ot[:, hf:])
                        else:
                            nc.sync.dma_start(out=y_slice, in_=ot[:])
    nc.compile()
    return nc


_NC = None


def _get_nc():
    global _NC
    if _NC is None:
        _NC = _build()
    return _NC


def _prep_inputs(x, kernels, b):
    bf16 = ml_dtypes.bfloat16
    xb = np.ascontiguousarray(x, dtype=np.float32).astype(bf16)
    # U_p[kw, i, o] = sum_kh G[p, kh] w[o, i, kh, kw]; layout [i, (p, kw, o)]
    G = np.array(
        [[1, 0, 0], [0.5, 0.5, 0.5], [0.5, -0.5, 0.5], [0, 0, 1]], np.float32
    )
    wk = np.asarray(kernels, dtype=np.float32)
    u = np.einsum("pk,oikw->ipwo", G, wk)  # [128, 4, 3, 256]
    utb = np.ascontiguousarray(u.reshape(C_IN, 12 * C_OUT)).astype(bf16)
    # bias [256] -> [128, 2]: column h holds b[h*128 : (h+1)*128]
    btb = np.ascontiguousarray(
        np.asarray(b, dtype=np.float32).reshape(2, 128).T
    )
    return xb, utb, btb


def build_in_maps(x, kernels, b):
    xb, utb, btb = _prep_inputs(x, kernels, b)
    return [
        {"xs": xb[i * N_PER : (i + 1) * N_PER], "ut": utb, "bt": btb}
        for i in range(N_CORES)
    ]


def kernel(x, kernels, b):
    nc = _get_nc()
    in_maps = build_in_maps(x, kernels, b)
    res = run_bass_kernel_spmd(nc, in_maps, core_ids=list(range(N_CORES)))
    out = np.concatenate(
        [r["y"].reshape(N_PER, C_OUT, H, W) for r in res.results], axis=0
    )
    return np.ascontiguousarray(out, dtype=np.float32)
